# revision 4
# baseline (speedup 1.0000x reference)
"""Causal single-head attention [4, 2048, 1024] on 8 TRN2 NeuronCores.

Sharding: pure SPMD, no collectives. core = 2*b + h  (b = batch, h = query
zigzag half). Each core owns 8 query tiles of 128 rows, zigzag-interleaved so
causal work is balanced: h=0 -> global q128-tiles [0,2,4,6,9,11,13,15],
h=1 -> [1,3,5,7,8,10,12,14] (both sum to 68 causal k-tile visits).

Per-core pipeline (all matmul operands bf16, PSUM f32):
  QT[f,q]   = Wq'^T x_q^T   (Wq' = Wq/32, scale folded on host; xTq fed by host)
  KT[f,k]   = Wk^T x^T      (full 2048 keys; duplicated across the pair)
  V[k,f]    = x Wv
  S^T[k,q]  = KT^T-contracted scores in transposed layout -> exp -> * causal
              mask (0/1, host-supplied data so both parities run ONE program)
  ctx[q,f]  = sum_k E[k,q] V[k,f] accumulated in PSUM; denominator via an
              extra N=1 matmul against a ones vector; divide by reciprocal.

The scores layout [k,q] makes E directly usable as the stationary operand of
the context matmul -- no transposes anywhere on device (host feeds x^T).
Softmax skips max-subtraction: scores = q.k/32 have |s| <~ 2, exp is safe.
"""

import os
import sys

os.environ.setdefault("JAX_PLATFORMS", "axon")
for _p in (
    "/root/.axon_site",
    "/root/.axon_site/_ro/trn_rl_repo",
    "/root/.axon_site/_ro/pypackages",
    "/opt/trn_rl_repo",
):
    if os.path.isdir(_p) and _p not in sys.path:
        sys.path.append(_p)

import ml_dtypes
import numpy as np

import concourse.bass as bass  # noqa: F401  (import keeps bass registered)
import concourse.tile as tile
from concourse import bacc, mybir
from concourse.bass_utils import run_bass_kernel_spmd

bf16 = ml_dtypes.bfloat16

B, S, D = 4, 2048, 1024
P = 128
NQT = S // P                  # 16 global q128 tiles per batch
N_CORES = 8
SCALE = 1.0 / 32.0            # 1/sqrt(1024)

# zigzag query-tile assignment: pair (j, 15-j), alternate between halves
GSEL = (
    [0, 2, 4, 6, 9, 11, 13, 15],   # h = 0
    [1, 3, 5, 7, 8, 10, 12, 14],   # h = 1
)
KJ = (8, 16)                   # scores k128-tile count per local q512 block
KAV = [2, 4, 6, 8, 10, 12, 14, 16]  # context k128-tile count per local q128
N_MASKS = sum(KJ)              # 24


def _masks_for(gsel: list[int]) -> np.ndarray:
    """[24,128,512] bf16 0/1 masks, scores layout [k=part, q=free]."""
    m = np.zeros((N_MASKS, P, 4 * P), dtype=np.float32)
    tri = (np.arange(P)[:, None] <= np.arange(P)[None, :]).astype(np.float32)
    mi = 0
    for J in range(2):
        for t in range(KJ[J]):
            for c in range(4):
                g = gsel[4 * J + c]
                if t < g:
                    m[mi, :, P * c:P * (c + 1)] = 1.0
                elif t == g:
                    m[mi, :, P * c:P * (c + 1)] = tri
            mi += 1
    return m.astype(bf16)


def _emit(nc, tc, reps=1):
    f32 = mybir.dt.float32
    bt = mybir.dt.bfloat16
    ND = D // P                # 8

    xt_d = nc.dram_tensor("xt", [D, S], bt, kind="ExternalInput").ap()
    xtq_d = nc.dram_tensor("xtq", [D, D], bt, kind="ExternalInput").ap()
    wq_d = nc.dram_tensor("wq", [D, D], bt, kind="ExternalInput").ap()
    wk_d = nc.dram_tensor("wk", [D, D], bt, kind="ExternalInput").ap()
    wv_d = nc.dram_tensor("wv", [D, D], bt, kind="ExternalInput").ap()
    mask_d = nc.dram_tensor("masks", [N_MASKS, P, 4 * P], bt, kind="ExternalInput").ap()
    out_d = nc.dram_tensor("out", [D, D], f32, kind="ExternalOutput").ap()

    for _rep in range(reps):
        _emit_once(nc, tc, xt_d, xtq_d, wq_d, wk_d, wv_d, mask_d, out_d)


def _emit_once(nc, tc, xt_d, xtq_d, wq_d, wk_d, wv_d, mask_d, out_d):
    f32 = mybir.dt.float32
    bt = mybir.dt.bfloat16
    ND = D // P                # 8

    with (
        tc.tile_pool(name="xtp", bufs=ND) as xtp,
        tc.tile_pool(name="ktp", bufs=ND) as ktp,
        tc.tile_pool(name="vp", bufs=S // P) as vp,
        tc.tile_pool(name="qtp", bufs=ND) as qtp,
        tc.tile_pool(name="ep", bufs=18) as ep,
        tc.tile_pool(name="mp", bufs=6) as mp,
        tc.tile_pool(name="op", bufs=4) as op,
        tc.tile_pool(name="smallp", bufs=1) as smallp,
    ):
        ones = smallp.tile([P, 1], bt, tag="ones")
        nc.vector.memset(ones[:], 1.0)

        qt = [qtp.tile([P, D], bt, tag="qt", name=f"qt{m}") for m in range(ND)]
        kt = [ktp.tile([P, S], bt, tag="kt", name=f"kt{m}") for m in range(ND)]
        vv = [vp.tile([P, D], bt, tag="v", name=f"v{k}") for k in range(S // P)]

        # ---- projections ----
        with (
            tc.tile_pool(name="wp", bufs=10) as wp,
            tc.tile_pool(name="xqp", bufs=ND) as xqp,
            tc.tile_pool(name="pp", bufs=4, space="PSUM") as pp,
        ):
            # DMA issue order matters: the first matmul group needs wq+xtq, so
            # issue those first; xt (4MB) would otherwise hog the queue and
            # stall the PE for ~20us at kernel start.
            xtq = []
            wqt = []
            for di in range(ND):
                t = wp.tile([P, D], bt, tag="w", name=f"wq{di}")
                nc.sync.dma_start(t[:], wq_d[P * di:P * (di + 1), :])
                wqt.append(t)
                t2 = xqp.tile([P, D], bt, tag="xq", name=f"xtq{di}")
                nc.sync.dma_start(t2[:], xtq_d[P * di:P * (di + 1), :])
                xtq.append(t2)

            xt = []
            for di in range(ND):
                t = xtp.tile([P, S], bt, tag="xt", name=f"xt{di}")
                nc.sync.dma_start(t[:], xt_d[P * di:P * (di + 1), :])
                xt.append(t)
            for m in range(ND):
                for qb in range(2):
                    ps = pp.tile([P, 512], f32, tag="ps", name="psq")
                    for di in range(ND):
                        nc.tensor.matmul(
                            ps[:],
                            wqt[di][:, P * m:P * (m + 1)],
                            xtq[di][:, 512 * qb:512 * (qb + 1)],
                            start=(di == 0), stop=(di == ND - 1),
                        )
                    nc.scalar.copy(qt[m][:, 512 * qb:512 * (qb + 1)], ps[:])

            # K^T[f, k] = sum_d Wk[d, f] xT[d, k]
            wkt = []
            for di in range(ND):
                t = wp.tile([P, D], bt, tag="w", name=f"wk{di}")
                nc.sync.dma_start(t[:], wk_d[P * di:P * (di + 1), :])
                wkt.append(t)
            for m in range(ND):
                for kb in range(S // 512):
                    ps = pp.tile([P, 512], f32, tag="ps", name="psk")
                    for di in range(ND):
                        nc.tensor.matmul(
                            ps[:],
                            wkt[di][:, P * m:P * (m + 1)],
                            xt[di][:, 512 * kb:512 * (kb + 1)],
                            start=(di == 0), stop=(di == ND - 1),
                        )
                    nc.scalar.copy(kt[m][:, 512 * kb:512 * (kb + 1)], ps[:])

            # V[k, f] = sum_d xT[d, k] Wv[d, f]
            wvt = []
            for di in range(ND):
                t = wp.tile([P, D], bt, tag="w", name=f"wv{di}")
                nc.sync.dma_start(t[:], wv_d[P * di:P * (di + 1), :])
                wvt.append(t)
            for k in range(S // P):
                for fh in range(2):
                    ps = pp.tile([P, 512], f32, tag="ps", name="psv")
                    for di in range(ND):
                        nc.tensor.matmul(
                            ps[:],
                            xt[di][:, P * k:P * (k + 1)],
                            wvt[di][:, 512 * fh:512 * (fh + 1)],
                            start=(di == 0), stop=(di == ND - 1),
                        )
                    nc.scalar.copy(vv[k][:, 512 * fh:512 * (fh + 1)], ps[:])

        # ---- attention ----
        with (
            tc.tile_pool(name="sp", bufs=2, space="PSUM") as sp,
            tc.tile_pool(name="cp", bufs=2, space="PSUM") as cp,
            tc.tile_pool(name="zp", bufs=2, space="PSUM") as zp,
            tc.tile_pool(name="rp", bufs=3) as rp,
        ):
            mi = 0
            for J in range(2):
                ee = []
                for t in range(KJ[J]):
                    ps = sp.tile([P, 512], f32, tag="sc", name="sc")
                    for fi in range(ND):
                        nc.tensor.matmul(
                            ps[:],
                            kt[fi][:, P * t:P * (t + 1)],
                            qt[fi][:, 512 * J:512 * (J + 1)],
                            start=(fi == 0), stop=(fi == ND - 1),
                        )
                    e = ep.tile([P, 512], bt, tag="e", name=f"e{J}_{t}")
                    nc.scalar.activation(e[:], ps[:], mybir.ActivationFunctionType.Exp)
                    mt = mp.tile([P, 512], bt, tag="m", name="mt")
                    nc.sync.dma_start(mt[:], mask_d[mi, :, :])
                    nc.vector.tensor_mul(e[:], e[:], mt[:])
                    ee.append(e)
                    mi += 1
                for c in range(4):
                    j = 4 * J + c
                    n = KAV[j]
                    ctx = cp.tile([P, D], f32, tag="ctx", name="ctx")
                    sm = zp.tile([P, 1], f32, tag="sm", name="sm")
                    for t in range(n):
                        lhs = ee[t][:, P * c:P * (c + 1)]
                        nc.tensor.matmul(ctx[:, 0:512], lhs, vv[t][:, 0:512],
                                         start=(t == 0), stop=(t == n - 1))
                        nc.tensor.matmul(ctx[:, 512:1024], lhs, vv[t][:, 512:1024],
                                         start=(t == 0), stop=(t == n - 1))
                        nc.tensor.matmul(sm[:], lhs, ones[:],
                                         start=(t == 0), stop=(t == n - 1))
                    rc = rp.tile([P, 1], f32, tag="rc", name="rc")
                    nc.vector.reciprocal(rc[:], sm[:])
                    o = op.tile([P, D], f32, tag="o", name="o")
                    nc.vector.tensor_scalar_mul(o[:], ctx[:], rc[:])
                    nc.sync.dma_start(out_d[P * j:P * (j + 1), :], o[:])


_CACHE = {}


def _build(reps=1):
    nc = bacc.Bacc(
        "TRN2", target_bir_lowering=False, debug=False,
        enable_asserts=False, num_devices=N_CORES,
    )
    with tile.TileContext(nc) as tc:
        _emit(nc, tc, reps=reps)
    nc.compile()
    return nc


def build_in_maps(x, W_query, W_key, W_value):
    wq = (np.asarray(W_query, np.float32) * SCALE).astype(bf16)
    wk = np.asarray(W_key, np.float32).astype(bf16)
    wv = np.asarray(W_value, np.float32).astype(bf16)
    masks = [_masks_for(GSEL[0]), _masks_for(GSEL[1])]
    in_maps = []
    for core in range(N_CORES):
        b, h = divmod(core, 2)
        xb = np.asarray(x[b], np.float32)
        qrows = np.concatenate([np.arange(P * g, P * (g + 1)) for g in GSEL[h]])
        in_maps.append({
            "xt": np.ascontiguousarray(xb.T).astype(bf16),
            "xtq": np.ascontiguousarray(xb[qrows].T).astype(bf16),
            "wq": wq, "wk": wk, "wv": wv,
            "masks": masks[h],
        })
    return in_maps


def assemble_out(results) -> np.ndarray:
    out = np.empty((B, S, D), dtype=np.float32)
    for core in range(N_CORES):
        b, h = divmod(core, 2)
        r = results[core]["out"]
        for j, g in enumerate(GSEL[h]):
            out[b, P * g:P * (g + 1), :] = r[P * j:P * (j + 1), :]
    return out


def kernel(x, W_query, W_key, W_value):
    if "nc" not in _CACHE:
        _CACHE["nc"] = _build()
    nc = _CACHE["nc"]
    in_maps = build_in_maps(x, W_query, W_key, W_value)
    r = run_bass_kernel_spmd(nc, in_maps, core_ids=list(range(N_CORES)))
    return assemble_out(r.results)


if __name__ == "__main__":
    rng = np.random.default_rng(0)
    x = rng.standard_normal((B, S, D), dtype=np.float32)
    bound = 1.0 / np.sqrt(D)
    wq = rng.uniform(-bound, bound, (D, D)).astype(np.float32)
    wk = rng.uniform(-bound, bound, (D, D)).astype(np.float32)
    wv = rng.uniform(-bound, bound, (D, D)).astype(np.float32)
    o = kernel(x, wq, wk, wv)
    print("out", o.shape, o.dtype, float(np.abs(o).max()))


# revision 5
# speedup vs baseline: 25.5210x; 25.5210x over previous
"""Causal single-head attention [4, 2048, 1024] on 8 TRN2 NeuronCores.

Sharding: pure SPMD, no collectives. core = 2*b + h  (b = batch, h = query
zigzag half). Each core owns 8 query tiles of 128 rows, zigzag-interleaved so
causal work is balanced: h=0 -> global q128-tiles [0,2,4,6,9,11,13,15],
h=1 -> [1,3,5,7,8,10,12,14] (both sum to 68 causal k-tile visits).

Per-core pipeline (all matmul operands bf16, PSUM f32):
  QT[f,q]   = Wq'^T x_q^T   (Wq' = Wq/32, scale folded on host; xTq fed by host)
  KT[f,k]   = Wk^T x^T      (full 2048 keys; duplicated across the pair)
  V[k,f]    = x Wv
  S^T[k,q]  = KT^T-contracted scores in transposed layout -> exp -> * causal
              mask (0/1, host-supplied data so both parities run ONE program)
  ctx[q,f]  = sum_k E[k,q] V[k,f] accumulated in PSUM; denominator via an
              extra N=1 matmul against a ones vector; divide by reciprocal.

The scores layout [k,q] makes E directly usable as the stationary operand of
the context matmul -- no transposes anywhere on device (host feeds x^T).
Softmax skips max-subtraction: scores = q.k/32 have |s| <~ 2, exp is safe.
"""

import os
import sys

os.environ.setdefault("JAX_PLATFORMS", "axon")
for _p in (
    "/root/.axon_site",
    "/root/.axon_site/_ro/trn_rl_repo",
    "/root/.axon_site/_ro/pypackages",
    "/opt/trn_rl_repo",
):
    if os.path.isdir(_p) and _p not in sys.path:
        sys.path.append(_p)

import ml_dtypes
import numpy as np

import concourse.bass as bass  # noqa: F401  (import keeps bass registered)
import concourse.tile as tile
from concourse import bacc, mybir
from concourse.bass_utils import run_bass_kernel_spmd

bf16 = ml_dtypes.bfloat16

B, S, D = 4, 2048, 1024
P = 128
NQT = S // P                  # 16 global q128 tiles per batch
N_CORES = 8
SCALE = 1.0 / 32.0            # 1/sqrt(1024)

# zigzag query-tile assignment: pair (j, 15-j), alternate between halves
GSEL = (
    [0, 2, 4, 6, 9, 11, 13, 15],   # h = 0
    [1, 3, 5, 7, 8, 10, 12, 14],   # h = 1
)
KJ = (8, 16)                   # scores k128-tile count per local q512 block
KAV = [2, 4, 6, 8, 10, 12, 14, 16]  # context k128-tile count per local q128
N_MASKS = sum(KJ)              # 24


def _masks_for(gsel: list[int]) -> np.ndarray:
    """[24,128,512] bf16 0/1 masks, scores layout [k=part, q=free]."""
    m = np.zeros((N_MASKS, P, 4 * P), dtype=np.float32)
    tri = (np.arange(P)[:, None] <= np.arange(P)[None, :]).astype(np.float32)
    mi = 0
    for J in range(2):
        for t in range(KJ[J]):
            for c in range(4):
                g = gsel[4 * J + c]
                if t < g:
                    m[mi, :, P * c:P * (c + 1)] = 1.0
                elif t == g:
                    m[mi, :, P * c:P * (c + 1)] = tri
            mi += 1
    return m.astype(bf16)


def _emit(nc, tc, reps=1):
    f32 = mybir.dt.float32
    bt = mybir.dt.bfloat16
    ND = D // P                # 8

    xt_d = nc.dram_tensor("xt", [D, S], bt, kind="ExternalInput").ap()
    xtq_d = nc.dram_tensor("xtq", [D, D], bt, kind="ExternalInput").ap()
    wq_d = nc.dram_tensor("wq", [D, D], bt, kind="ExternalInput").ap()
    wk_d = nc.dram_tensor("wk", [D, D], bt, kind="ExternalInput").ap()
    wv_d = nc.dram_tensor("wv", [D, D], bt, kind="ExternalInput").ap()
    mask_d = nc.dram_tensor("masks", [N_MASKS, P, 4 * P], bt, kind="ExternalInput").ap()
    out_d = nc.dram_tensor("out", [D, D], f32, kind="ExternalOutput").ap()

    for _rep in range(reps):
        _emit_once(nc, tc, xt_d, xtq_d, wq_d, wk_d, wv_d, mask_d, out_d)


def _emit_once(nc, tc, xt_d, xtq_d, wq_d, wk_d, wv_d, mask_d, out_d):
    f32 = mybir.dt.float32
    bt = mybir.dt.bfloat16
    ND = D // P                # 8

    with (
        tc.tile_pool(name="xtp", bufs=ND) as xtp,
        tc.tile_pool(name="ktp", bufs=ND) as ktp,
        tc.tile_pool(name="vp", bufs=S // P) as vp,
        tc.tile_pool(name="qtp", bufs=ND) as qtp,
        tc.tile_pool(name="ep", bufs=18) as ep,
        tc.tile_pool(name="mp", bufs=6) as mp,
        tc.tile_pool(name="op", bufs=4) as op,
        tc.tile_pool(name="smallp", bufs=1) as smallp,
    ):
        ones = smallp.tile([P, 1], bt, tag="ones")
        nc.vector.memset(ones[:], 1.0)

        qt = [qtp.tile([P, D], bt, tag="qt", name=f"qt{m}") for m in range(ND)]
        kt = [ktp.tile([P, S], bt, tag="kt", name=f"kt{m}") for m in range(ND)]
        vv = [vp.tile([P, D], bt, tag="v", name=f"v{k}") for k in range(S // P)]

        # ---- projections ----
        with (
            tc.tile_pool(name="wp", bufs=10) as wp,
            tc.tile_pool(name="xqp", bufs=ND) as xqp,
            tc.tile_pool(name="pp", bufs=4, space="PSUM") as pp,
        ):
            # DMA issue order matters: the first matmul group needs wq+xtq, so
            # issue those first; xt (4MB) would otherwise hog the queue and
            # stall the PE for ~20us at kernel start.
            xtq = []
            wqt = []
            for di in range(ND):
                t = wp.tile([P, D], bt, tag="w", name=f"wq{di}")
                nc.sync.dma_start(t[:], wq_d[P * di:P * (di + 1), :])
                wqt.append(t)
                t2 = xqp.tile([P, D], bt, tag="xq", name=f"xtq{di}")
                nc.sync.dma_start(t2[:], xtq_d[P * di:P * (di + 1), :])
                xtq.append(t2)

            xt = []
            for di in range(ND):
                t = xtp.tile([P, S], bt, tag="xt", name=f"xt{di}")
                nc.sync.dma_start(t[:], xt_d[P * di:P * (di + 1), :])
                xt.append(t)
            for m in range(ND):
                for qb in range(2):
                    ps = pp.tile([P, 512], f32, tag="ps", name="psq")
                    for di in range(ND):
                        nc.tensor.matmul(
                            ps[:],
                            wqt[di][:, P * m:P * (m + 1)],
                            xtq[di][:, 512 * qb:512 * (qb + 1)],
                            start=(di == 0), stop=(di == ND - 1),
                        )
                    nc.vector.tensor_copy(qt[m][:, 512 * qb:512 * (qb + 1)], ps[:])

            # K^T[f, k] = sum_d Wk[d, f] xT[d, k]
            wkt = []
            for di in range(ND):
                t = wp.tile([P, D], bt, tag="w", name=f"wk{di}")
                nc.sync.dma_start(t[:], wk_d[P * di:P * (di + 1), :])
                wkt.append(t)
            for m in range(ND):
                for kb in range(S // 512):
                    ps = pp.tile([P, 512], f32, tag="ps", name="psk")
                    for di in range(ND):
                        nc.tensor.matmul(
                            ps[:],
                            wkt[di][:, P * m:P * (m + 1)],
                            xt[di][:, 512 * kb:512 * (kb + 1)],
                            start=(di == 0), stop=(di == ND - 1),
                        )
                    nc.vector.tensor_copy(kt[m][:, 512 * kb:512 * (kb + 1)], ps[:])

            # V[k, f] = sum_d xT[d, k] Wv[d, f]
            wvt = []
            for di in range(ND):
                t = wp.tile([P, D], bt, tag="w", name=f"wv{di}")
                nc.sync.dma_start(t[:], wv_d[P * di:P * (di + 1), :])
                wvt.append(t)
            for k in range(S // P):
                for fh in range(2):
                    ps = pp.tile([P, 512], f32, tag="ps", name="psv")
                    for di in range(ND):
                        nc.tensor.matmul(
                            ps[:],
                            xt[di][:, P * k:P * (k + 1)],
                            wvt[di][:, 512 * fh:512 * (fh + 1)],
                            start=(di == 0), stop=(di == ND - 1),
                        )
                    nc.vector.tensor_copy(vv[k][:, 512 * fh:512 * (fh + 1)], ps[:])

        # ---- attention ----
        with (
            tc.tile_pool(name="sp", bufs=2, space="PSUM") as sp,
            tc.tile_pool(name="cp", bufs=2, space="PSUM") as cp,
            tc.tile_pool(name="zp", bufs=2, space="PSUM") as zp,
            tc.tile_pool(name="rp", bufs=3) as rp,
        ):
            mi = 0
            for J in range(2):
                ee = []
                for t in range(KJ[J]):
                    ps = sp.tile([P, 512], f32, tag="sc", name="sc")
                    for fi in range(ND):
                        nc.tensor.matmul(
                            ps[:],
                            kt[fi][:, P * t:P * (t + 1)],
                            qt[fi][:, 512 * J:512 * (J + 1)],
                            start=(fi == 0), stop=(fi == ND - 1),
                        )
                    e = ep.tile([P, 512], bt, tag="e", name=f"e{J}_{t}")
                    nc.scalar.activation(e[:], ps[:], mybir.ActivationFunctionType.Exp)
                    mt = mp.tile([P, 512], bt, tag="m", name="mt")
                    nc.sync.dma_start(mt[:], mask_d[mi, :, :])
                    nc.vector.tensor_mul(e[:], e[:], mt[:])
                    ee.append(e)
                    mi += 1
                for c in range(4):
                    j = 4 * J + c
                    n = KAV[j]
                    ctx = cp.tile([P, D], f32, tag="ctx", name="ctx")
                    sm = zp.tile([P, 1], f32, tag="sm", name="sm")
                    for t in range(n):
                        lhs = ee[t][:, P * c:P * (c + 1)]
                        nc.tensor.matmul(ctx[:, 0:512], lhs, vv[t][:, 0:512],
                                         start=(t == 0), stop=(t == n - 1))
                        nc.tensor.matmul(ctx[:, 512:1024], lhs, vv[t][:, 512:1024],
                                         start=(t == 0), stop=(t == n - 1))
                        nc.tensor.matmul(sm[:], lhs, ones[:],
                                         start=(t == 0), stop=(t == n - 1))
                    rc = rp.tile([P, 1], f32, tag="rc", name="rc")
                    nc.vector.reciprocal(rc[:], sm[:])
                    o = op.tile([P, D], f32, tag="o", name="o")
                    nc.vector.tensor_scalar_mul(o[:], ctx[:], rc[:])
                    nc.sync.dma_start(out_d[P * j:P * (j + 1), :], o[:])


_CACHE = {}


def _build(reps=1):
    nc = bacc.Bacc(
        "TRN2", target_bir_lowering=False, debug=False,
        enable_asserts=False, num_devices=N_CORES,
    )
    with tile.TileContext(nc) as tc:
        _emit(nc, tc, reps=reps)
    nc.compile()
    return nc


def build_in_maps(x, W_query, W_key, W_value):
    wq = (np.asarray(W_query, np.float32) * SCALE).astype(bf16)
    wk = np.asarray(W_key, np.float32).astype(bf16)
    wv = np.asarray(W_value, np.float32).astype(bf16)
    masks = [_masks_for(GSEL[0]), _masks_for(GSEL[1])]
    in_maps = []
    for core in range(N_CORES):
        b, h = divmod(core, 2)
        xb = np.asarray(x[b], np.float32)
        qrows = np.concatenate([np.arange(P * g, P * (g + 1)) for g in GSEL[h]])
        in_maps.append({
            "xt": np.ascontiguousarray(xb.T).astype(bf16),
            "xtq": np.ascontiguousarray(xb[qrows].T).astype(bf16),
            "wq": wq, "wk": wk, "wv": wv,
            "masks": masks[h],
        })
    return in_maps


def assemble_out(results) -> np.ndarray:
    out = np.empty((B, S, D), dtype=np.float32)
    for core in range(N_CORES):
        b, h = divmod(core, 2)
        r = results[core]["out"]
        for j, g in enumerate(GSEL[h]):
            out[b, P * g:P * (g + 1), :] = r[P * j:P * (j + 1), :]
    return out


def kernel(x, W_query, W_key, W_value):
    if "nc" not in _CACHE:
        _CACHE["nc"] = _build()
    nc = _CACHE["nc"]
    in_maps = build_in_maps(x, W_query, W_key, W_value)
    r = run_bass_kernel_spmd(nc, in_maps, core_ids=list(range(N_CORES)))
    return assemble_out(r.results)


if __name__ == "__main__":
    rng = np.random.default_rng(0)
    x = rng.standard_normal((B, S, D), dtype=np.float32)
    bound = 1.0 / np.sqrt(D)
    wq = rng.uniform(-bound, bound, (D, D)).astype(np.float32)
    wk = rng.uniform(-bound, bound, (D, D)).astype(np.float32)
    wv = rng.uniform(-bound, bound, (D, D)).astype(np.float32)
    o = kernel(x, wq, wk, wv)
    print("out", o.shape, o.dtype, float(np.abs(o).max()))
